# revision 1
# baseline (speedup 1.0000x reference)
"""Trainium2 Bass kernel for nn_Net_9655086481488 (IndRNN encoder/decoder).

Mathematical reduction (exact, holds for any input values):
  - reference takes y[:, -1] after the encoder: only batch element B-1 of the
    encoder output is used.
  - it then takes out[:, 0] after the decoder, whose batch dim is the encoder
    TIME dim: only encoder timestep 0 survives.
  - the IndRNN scan starts from h0 = 0, so timestep 0 of each encoder layer is
    just relu(W @ x_0 + b) -- no recurrence needed.
  => predict depends only on v = x[0, B-1, :] (2 floats):
       h1   = relu(enc_w0 @ v + enc_b0)                  (1024,)
       h2   = relu(enc_w1 @ h1 + enc_b1)                 (1024,)
       p0   = dec_w0 @ h2 + dec_b0                       (1024,)  const over p
       g_p  = relu(p0 + dec_u0 * g_{p-1})                20-step scan
            = relu(p0) * a_p   with a_p = max(dec_u0 * a_{p-1} + 1, 0), a_0 = 1
       pre2 = G @ dec_w1.T + dec_b1                      (20, 1024)
       o_p  = relu(pre2_p + dec_u1 * o_{p-1})            20-step scan
       predict = O @ out_w.T + out_b                     (20, 2)

Sharding over 8 cores: enc_w1 / dec_w0 replicated (full vectors needed for the
nonlinear chain); dec_w1 / out_w / dec_u1 / dec_b1 sharded by 128 hidden lanes
per core. Each core returns a (20, 2) partial of the output head; the host sum
of the 8 partials (+ out_b) is the gather/unshard step. No collectives.

The three 1024x1024 weight matrices ship as float16 (the decoder scans amplify
rounding ~30x on |u|~1 lanes: bf16 lands at 2.8e-2 rel err, fp16 at 9.8e-4).
Weights are pre-transposed and pre-tiled on the host so every DMA is fully
contiguous per partition and every matmul consumes natural [K, M] layouts.
"""

import numpy as np

T = 20          # encoder timesteps
P = 20          # predict steps
B = 4096
H = 1024
NCORES = 8
HC = H // NCORES  # 128 hidden lanes per core / per chunk
KC = H // 128     # 8 k-chunks of 128
NSLAB = 4         # dma slabs per big matrix (KC % NSLAB == 0)

# small-pack column layout (one (128, 56) f32 tile per core)
C_U0 = 0      # cols 0-7:   dec_u0   chunk-major
C_EB1 = 8     # cols 8-15:  enc_b1
C_DB0 = 16    # cols 16-23: dec_b0
C_EB0 = 24    # cols 24-31: enc_b0
C_U1 = 32     # col 32:     dec_u1 shard (this core's 128 lanes)
C_DB1 = 33    # col 33:     dec_b1 shard
C_W00 = 34    # cols 34-41: enc_w0[:, 0] chunk-major
C_W01 = 42    # cols 42-49: enc_w0[:, 1] chunk-major
C_V0 = 50     # col 50:     x[0, B-1, 0] replicated
C_V1 = 51     # col 51:     x[0, B-1, 1] replicated
C_OW = 52     # cols 52-53: out_w.T shard (128, 2)
C_OWN = 54    # cols 54-55: negated out_w.T shard
NSMALL = 56

_CACHE = {}


def _build():
    import concourse.mybir as mybir
    from concourse import bacc, tile

    f32 = mybir.dt.float32
    f16 = mybir.dt.float16
    Relu = mybir.ActivationFunctionType.Relu
    mult = mybir.AluOpType.mult
    add = mybir.AluOpType.add
    sub = mybir.AluOpType.subtract
    amax = mybir.AluOpType.max

    nc = bacc.Bacc("TRN2", target_bir_lowering=False, debug=False,
                   num_devices=NCORES)

    small_h = nc.dram_tensor("small", [128, NSMALL], f32, kind="ExternalInput")
    # pre-tiled on host: [p, kc, m] = W.T[kc*128 + p, m], fully contiguous
    ew1T_h = nc.dram_tensor("ew1T", [128, KC, H], f16, kind="ExternalInput")
    dw0T_h = nc.dram_tensor("dw0T", [128, KC, H], f16, kind="ExternalInput")
    dw1c_h = nc.dram_tensor("dw1c", [128, KC, HC], f16, kind="ExternalInput")
    out_h = nc.dram_tensor("out", [P, 2], f32, kind="ExternalOutput")

    with tile.TileContext(nc) as tc:
        with (
            tc.tile_pool(name="w", bufs=1) as wpool,
            tc.tile_pool(name="s", bufs=1) as spool,
            tc.tile_pool(name="tmp", bufs=2) as tpool,
            tc.tile_pool(name="psum", bufs=1, space="PSUM") as ppool,
        ):
            smallt = wpool.tile([128, NSMALL], f32, tag="small")
            ew1t = wpool.tile([128, KC, H], f16, tag="ew1")
            dw0t = wpool.tile([128, KC, H], f16, tag="dw0")
            dw1t = wpool.tile([128, KC, HC], f16, tag="dw1")

            h1s = spool.tile([128, KC], f16, tag="h1")
            h2s = spool.tile([128, KC], f16, tag="h2")
            rp0s = spool.tile([128, KC], f32, tag="rp0")
            Ast = spool.tile([128, KC, P], f32, tag="Ast")
            gt = spool.tile([128, KC, P], f16, tag="gt")
            pre2b = spool.tile([128, P], f32, tag="pre2b")

            # ---- DMAs: ew1 slabs first, then dw1c, then dw0 slabs ----
            CPS = KC // NSLAB  # chunks per slab
            nc.sync.dma_start(out=ew1t[:, 0:CPS, :], in_=ew1T_h.ap()[:, 0:CPS, :])
            nc.sync.dma_start(out=smallt[:, :], in_=small_h.ap())
            for s in range(1, NSLAB):
                sl = slice(s * CPS, (s + 1) * CPS)
                nc.sync.dma_start(out=ew1t[:, sl, :], in_=ew1T_h.ap()[:, sl, :])
            lo = 0
            for w in (3, 3, 1, 1):
                sl = slice(lo, lo + w)
                nc.sync.dma_start(out=dw0t[:, sl, :], in_=dw0T_h.ap()[:, sl, :])
                lo += w
            nc.sync.dma_start(out=dw1t[:, :, :], in_=dw1c_h.ap())

            # ---- A-scan: a_0 = 1, a_t = max(u0 * a_{t-1} + 1, 0) ----
            # (only depends on the small pack -> runs during the weight DMA)
            u0 = smallt[:, C_U0:C_U0 + KC]
            nc.vector.memset(Ast[:, :, 0], 1.0)
            for t in range(1, P):
                atmp = tpool.tile([128, KC], f32, tag="atmp")
                nc.vector.tensor_tensor(atmp[:, :], Ast[:, :, t - 1], u0, mult)
                nc.vector.tensor_scalar(Ast[:, :, t], atmp[:, :], 1.0, 0.0,
                                        add, amax)

            # ---- h1 = relu(w0c0*v0 + w0c1*v1 + enc_b0) on DVE ----
            t1 = tpool.tile([128, KC], f32, tag="h1a")
            t2 = tpool.tile([128, KC], f32, tag="h1b")
            nc.vector.tensor_scalar(t1[:, :], smallt[:, C_W00:C_W00 + KC],
                                    smallt[:, C_V0:C_V0 + 1], None, mult)
            nc.vector.tensor_scalar(t2[:, :], smallt[:, C_W01:C_W01 + KC],
                                    smallt[:, C_V1:C_V1 + 1], None, mult)
            nc.vector.tensor_tensor(t1[:, :], t1[:, :], t2[:, :], add)
            nc.vector.tensor_tensor(t1[:, :], t1[:, :],
                                    smallt[:, C_EB0:C_EB0 + KC], add)
            nc.vector.tensor_scalar(h1s[:, :], t1[:, :], 0.0, None, amax)

            # ---- h2 = relu(enc_w1 @ h1 + enc_b1) ----
            # kc-outer with 7 parallel accumulators + second pass for chunk 7,
            # so the last-arriving weight slab gates only ~8 matmul pairs
            NACC = KC - 1
            pms = [ppool.tile([128, 1], f32, tag="mv", bufs=7, name=f"pm{i}")
                   for i in range(NACC)]
            for kc in range(KC):
                for mc in range(NACC):
                    nc.tensor.matmul(pms[mc][:, :],
                                     ew1t[:, kc, mc * 128:(mc + 1) * 128],
                                     h1s[:, kc:kc + 1],
                                     start=(kc == 0), stop=(kc == KC - 1))
            pm7 = ppool.tile([128, 1], f32, tag="pp")
            for kc in range(KC):
                nc.tensor.matmul(pm7[:, :],
                                 ew1t[:, kc, NACC * 128:KC * 128],
                                 h1s[:, kc:kc + 1],
                                 start=(kc == 0), stop=(kc == KC - 1))
            for mc in range(KC):
                pm = pm7 if mc == NACC else pms[mc]
                nc.scalar.activation(h2s[:, mc:mc + 1], pm[:, :], Relu,
                                     bias=smallt[:, C_EB1 + mc:C_EB1 + mc + 1])

            # ---- rp0 = relu(dec_w0 @ h2 + dec_b0); G^T = rp0 * A ----
            # kc-outer so the last-arriving dec_w0 slab gates few matmuls
            pjs = [ppool.tile([128, 1], f32, tag="mv", bufs=7, name=f"pj{i}")
                   for i in range(NACC)]
            for kc in range(KC):
                for jc in range(NACC):
                    nc.tensor.matmul(pjs[jc][:, :],
                                     dw0t[:, kc, jc * 128:(jc + 1) * 128],
                                     h2s[:, kc:kc + 1],
                                     start=(kc == 0), stop=(kc == KC - 1))
            pj7 = ppool.tile([128, 1], f32, tag="pp")
            for kc in range(KC):
                nc.tensor.matmul(pj7[:, :],
                                 dw0t[:, kc, NACC * 128:KC * 128],
                                 h2s[:, kc:kc + 1],
                                 start=(kc == 0), stop=(kc == KC - 1))
            for jc in range(KC):
                pj = pj7 if jc == NACC else pjs[jc]
                nc.scalar.activation(rp0s[:, jc:jc + 1], pj[:, :], Relu,
                                     bias=smallt[:, C_DB0 + jc:C_DB0 + jc + 1])
            nc.vector.tensor_tensor(gt[:, :, :], Ast[:, :, :],
                                    rp0s[:, :].broadcast_to([128, KC, P]),
                                    mult)

            # ---- pre2^T = dec_w1_shard @ G + dec_b1_shard  (128 j, 20 t) ----
            pp = ppool.tile([128, P], f32, tag="pp")
            for kc in range(KC):
                nc.tensor.matmul(pp[:, :], dw1t[:, kc, :], gt[:, kc, :],
                                 start=(kc == 0), stop=(kc == KC - 1))
            nc.vector.tensor_scalar(pre2b[:, :], pp[:, :],
                                    smallt[:, C_DB1:C_DB1 + 1], None, add)

            # ---- scan2: o_t = relu(pre2_t + u1 * o_{t-1}) via two HW scans
            # with the shift g_t = u*g_{t-1} - c_t:
            #   q_t = max(u*q_{t-1}, g_t)  and  o_t = q_t - g_t  (exact)
            u1b = smallt[:, C_U1:C_U1 + 1].broadcast_to([HC, P])
            gam = spool.tile([HC, P], f32, tag="gam")
            qsc = spool.tile([HC, P], f32, tag="qsc")
            nc.vector.tensor_tensor_scan(gam[:, :], u1b, pre2b[:, :], 0.0,
                                         mult, sub)
            nc.vector.tensor_tensor_scan(qsc[:, :], u1b, gam[:, :], 0.0,
                                         mult, amax)

            # ---- head partial: (20,2) = q.T @ ow - gam.T @ ow  (o = q-gam)
            hp = ppool.tile([P, 2], f32, tag="pp")
            nc.tensor.matmul(hp[:, :], qsc[:, :], smallt[:, C_OW:C_OW + 2],
                             start=True, stop=False)
            nc.tensor.matmul(hp[:, :], gam[:, :], smallt[:, C_OWN:C_OWN + 2],
                             start=False, stop=True)
            outs = spool.tile([P, 2], f32, tag="outs")
            nc.vector.tensor_copy(outs[:, :], hp[:, :])
            nc.sync.dma_start(out=out_h.ap(), in_=outs[:, :])

    nc.compile()
    return nc


def _chunk_major(vec):
    # vec (1024,) -> (128, 8) with [p, c] = vec[c*128 + p]
    return np.ascontiguousarray(vec.reshape(KC, 128).T)


def _tile_f16(wT):
    # W.T (1024, m) f32 -> (128, KC, m) f16 with [p, kc, m] = W.T[kc*128+p, m]
    return np.ascontiguousarray(
        wT.astype(np.float16).reshape(KC, 128, wT.shape[1]).transpose(1, 0, 2))


def kernel(x, enc_w0, enc_u0, enc_b0, enc_w1, enc_u1, enc_b1,
           dec_w0, dec_u0, dec_b0, dec_w1, dec_u1, dec_b1,
           out_w, out_b):
    import os
    from concourse.bass_utils import run_bass_kernel_spmd

    if "nc" not in _CACHE:
        _CACHE["nc"] = _build()
    nc = _CACHE["nc"]

    f = np.float32
    v = np.asarray(x, f)[0, -1, :]                              # (2,)
    ew0 = np.asarray(enc_w0, f)                                 # (1024, 2)
    ew1tiled = _tile_f16(np.asarray(enc_w1, f).T)               # (128, 8, 1024)
    dw0tiled = _tile_f16(np.asarray(dec_w0, f).T)               # (128, 8, 1024)
    dw1T = np.asarray(dec_w1, f).T                              # (1024, 1024)
    owT = np.asarray(out_w, f).T                                # (1024, 2)

    base = np.zeros((128, NSMALL), f)
    base[:, C_U0:C_U0 + KC] = _chunk_major(np.asarray(dec_u0, f))
    base[:, C_EB1:C_EB1 + KC] = _chunk_major(np.asarray(enc_b1, f))
    base[:, C_DB0:C_DB0 + KC] = _chunk_major(np.asarray(dec_b0, f))
    base[:, C_EB0:C_EB0 + KC] = _chunk_major(np.asarray(enc_b0, f))
    base[:, C_W00:C_W00 + KC] = _chunk_major(np.ascontiguousarray(ew0[:, 0]))
    base[:, C_W01:C_W01 + KC] = _chunk_major(np.ascontiguousarray(ew0[:, 1]))
    base[:, C_V0] = v[0]
    base[:, C_V1] = v[1]

    in_maps = []
    for c in range(NCORES):
        jsl = slice(c * HC, (c + 1) * HC)
        small = base.copy()
        small[:, C_U1] = np.asarray(dec_u1, f)[jsl]
        small[:, C_DB1] = np.asarray(dec_b1, f)[jsl]
        small[:, C_OW:C_OW + 2] = owT[jsl, :]
        small[:, C_OWN:C_OWN + 2] = -owT[jsl, :]
        in_maps.append({
            "small": small,
            "ew1T": ew1tiled,
            "dw0T": dw0tiled,
            "dw1c": _tile_f16(np.ascontiguousarray(dw1T[:, jsl])),
        })

    trace = bool(os.environ.get("KERNEL_TRACE"))
    res = run_bass_kernel_spmd(nc, in_maps, core_ids=list(range(NCORES)),
                               trace=trace)
    _CACHE["last_result"] = res
    partials = [res.results[c]["out"] for c in range(NCORES)]
    return (np.sum(partials, axis=0) + np.asarray(out_b, f)).astype(f)

